# revision 24
# baseline (speedup 1.0000x reference)
"""TRN2 Bass kernel for nn_Attention_15590731285136.

Computation (per batch b):
    g      = diag(W) * K[b]                                # [d]
    score  = relu(V[b] @ (g[:,None]*w1) + b1) @ w2 + b2    # [h]
    score  = where(mask[b], MASK_FILL, score)
    alpha  = softmax(score)                                # over h
    out[b] = alpha @ V[b]                                  # [d]

Sharding: data-parallel over batch, 8 batches per core on 8 NeuronCores.

Key transformations:
  * Masked tokens are dead weight: the host compacts each batch to its
    unmasked tokens (~half of 2048), padded to a multiple of 128; pad slots
    carry V=0 and a MASK_FILL additive bias.
  * The elementwise gate and |w2| fold into the weight matrix:
    z = V_s @ (g*w1*|w2|*SW) runs in fp8-e4m3 DoubleRow (2 rows/pass).
  * Score via the abs decomposition:
        score = 1/2 * sum_n w2_n*pre_n  +  1/2*(sum_pos|z| - sum_neg|z|)/s
    The linear half-term is computed EXACTLY on the host and folded into the
    additive mask-bias tensor, so the device only needs two |.|-sums per
    token tile (positive-w2 block, negative-w2 block, pre-permuted
    contiguous).  Each half runs as ONE fused instruction: ACT
    activation(Abs, accum_out) or DVE tensor_reduce(abs) (no scratch write,
    no read-accumulator on the DVE path); halves are statically assigned to
    whichever engine has less queued work.
  * Token-major V for the weighted sum is stored in DRAM as fp8-e3m4
    (x2 scale; 4 mantissa bits) and upcast to fp16 during the DMA itself
    (SWDGE cast) — half the HBM traffic, fp16 on-chip arithmetic.
  * Pass 2 (alpha @ V) runs on the PE as chained [128,1]x[128,512] matmuls,
    deferred by one batch so the PE never waits on scores; the last NDVE
    tiles per batch run on DVE instead (scalar_tensor_tensor chain).
  * softmax skips max-subtraction (scores are O(0.1); pad entries get an
    additive -2^32 bias so exp underflows to exactly 0); normalization
    (division by sum(alpha)) happens on the host on the [b, 512] outputs.
"""

import numpy as np

B, H, D, HID = 64, 2048, 512, 512
NCORES = 8
BPC = B // NCORES          # batches per core
DC = D // 128              # 4 contraction chunks
MASK_FILL = -2.0**32 + 1.0

SV = 16.0                  # fp8-e4m3 scale on V^T (|V| ~ N(0,1), e4m3 max 240)
SVN = 2.0                  # fp8-e3m4 scale on token-major V (e3m4 max 15.5)
WTARGET = 96.0             # target max |w12 * SW| after scaling
NDVE = 2                   # pass-2 tiles per batch computed on DVE


def _build(hcs, hp, b2eff, has_bias, escale2):
    import concourse.mybir as mybir
    from concourse import bacc
    from concourse.tile import TileContext

    F32 = mybir.dt.float32
    F16 = mybir.dt.float16
    F8 = mybir.dt.float8e4
    F8N = mybir.dt.float8e3
    ACTF = mybir.ActivationFunctionType
    ALU = mybir.AluOpType
    AXL = mybir.AxisListType
    DR = mybir.MatmulPerfMode.DoubleRow

    hc = max(hcs)               # tiles/dram sized for the largest slot
    ndve = min(NDVE, min(hcs))
    nc = bacc.Bacc(trn_type="TRN2", num_devices=NCORES)

    # VW layout per batch: [w3 (DC*HID) | vt tiles (hc*DC*128)] so the
    # weights + first token tiles can land in a small leading DMA.
    VWLEN = DC * HID + hc * DC * 128
    VW = nc.dram_tensor("VW", (BPC, 128, VWLEN), F8, kind="ExternalInput")
    VN = nc.dram_tensor("VN", (BPC, 128, hc * D), F8N, kind="ExternalInput")
    MB = nc.dram_tensor("MB", (BPC, 128, hc), F32, kind="ExternalInput")
    if has_bias:
        BI = nc.dram_tensor("BI", (1, HID), F32, kind="ExternalInput")
    OUT = nc.dram_tensor("OUT", (BPC, D + hc), F32, kind="ExternalOutput")

    # static engine-balance tracker (ns estimates incl. per-op overheads)
    eng_t = {"act": 0.0, "dve": 0.0}

    def act_half_cost(w):
        return (172 + w) / 1.2 + 291.0     # ACTIVATE(psum) + READ_ACC

    def dve_half_cost(w):
        return (120 + w) / 0.96            # TENSOR_REDUCE(abs, psum)

    with TileContext(nc) as tc:
        with (
            tc.tile_pool(name="const", bufs=1) as cpool,
            tc.tile_pool(name="w3p", bufs=4) as w3pool,
            tc.tile_pool(name="vt1", bufs=4) as vt1pool,
            tc.tile_pool(name="vt2", bufs=4) as vt2pool,
            tc.tile_pool(name="vn", bufs=3) as vnpool,
            tc.tile_pool(name="small", bufs=4) as spool,
            tc.tile_pool(name="scr", bufs=3) as scrpool,
            tc.tile_pool(name="fc1_ps", bufs=5, space="PSUM") as fc1ps,
            tc.tile_pool(name="tot_ps", bufs=1, space="PSUM") as totps,
            tc.tile_pool(name="acc_ps", bufs=2, space="PSUM") as accps,
        ):
            # ---- one-time constants ----
            ones_col = cpool.tile([128, 1], F16, tag="ones")
            nc.vector.memset(ones_col, 1.0)
            if has_bias:
                ones_row = cpool.tile([1, 128], F16, tag="orr")
                nc.vector.memset(ones_row, 1.0)
                bias_sb = cpool.tile([1, HID], F16, tag="bias")
                bias_f = cpool.tile([1, HID], F32, tag="biasf")
                nc.sync.dma_start(out=bias_f, in_=BI.ap())
                nc.vector.tensor_copy(bias_sb, bias_f)

            # staging tile for outputs; per-batch stores
            oball = cpool.tile([1, BPC * (D + hc)], F32, tag="oball")
            nc.vector.memset(
                oball.rearrange("o (b x) -> o b x", b=BPC)[:, :, D:], 0.0)

            NB1 = 2   # token tiles in the leading vt DMA

            def emit_vw(bi):
                # three independent DMAs: weights, first NB1 token tiles,
                # remaining token tiles — so fc1 can start on a ~0.4MB load
                hcb = hcs[bi]
                n1 = min(NB1, hcb)
                wa = w3pool.tile([128, DC * HID], F8, tag="wa")
                nc.sync.dma_start(out=wa, in_=VW.ap()[bi][:, :DC * HID])
                vb1 = vt1pool.tile([128, NB1 * DC * 128], F8, tag="vb1")
                nc.sync.dma_start(
                    out=vb1[:, :n1 * DC * 128],
                    in_=VW.ap()[bi][:, DC * HID:DC * HID + n1 * DC * 128])
                vb2 = vt2pool.tile([128, (hc - NB1) * DC * 128], F8, tag="vb2")
                if hcb > n1:
                    nc.sync.dma_start(
                        out=vb2[:, :(hcb - n1) * DC * 128],
                        in_=VW.ap()[bi][:, DC * HID + n1 * DC * 128:
                                        DC * HID + hcb * DC * 128])
                return (wa, vb1, vb2)

            def emit_vn(bi, gate=None):
                # fp8-e3m4 straight into SBUF; pass-2 matmuls read it as a
                # mixed-dtype rhs (fp16 alpha x fp8e3 V upconverts exactly).
                # A tiny DVE copy READING live compute data and writing into
                # the DMA's own region paces the transfer via the WAW dep, so
                # it cannot race ahead and starve the critical vt loads.
                li = hcs[bi] * D
                vn = vnpool.tile([128, hc * D], F8N, tag="vn")
                if gate is not None:
                    nc.vector.tensor_copy(vn[0:1, 0:1], gate[0:1, 0:1])
                nc.gpsimd.dma_start(out=vn[:, :li], in_=VN.ap()[bi][:, :li])
                return vn

            pend_vw = [emit_vw(0), emit_vw(1)] if BPC > 1 else [emit_vw(0)]

            # ---- all batches' additive score-bias columns in one DMA ----
            mall = cpool.tile([128, BPC * hc], F32, tag="mall")
            nc.sync.dma_start(
                out=mall.rearrange("p (b j) -> p b j", b=BPC),
                in_=MB.ap().rearrange("b p j -> p b j"),
            )

            defer1 = None   # batch bi-1: (alpha, dacc, vn, bi, hcb, ndve)
            defer2 = None   # batch bi-2: (pacc, dacc, bi, hcb, ndve) open

            for bi in range(BPC):
                if bi > 0 and bi + 2 < BPC:
                    pend_vw.append(emit_vw(bi + 2))
                hcb = hcs[bi]
                wa, vb1, vb2 = pend_vw.pop(0)
                if bi > 0:
                    vn = emit_vn(bi, gate=prev_sp)
                else:
                    vn = None   # emitted mid-loop, gated on batch-0 compute
                vt41 = vb1.rearrange("p (j c m) -> p j c m", j=NB1, c=DC)
                vt42 = vb2.rearrange("p (j c m) -> p j c m", j=hc - NB1, c=DC)
                w3 = wa.rearrange("p (c n) -> p c n", c=DC)
                mb = mall[:, bi * hc:bi * hc + hcb]

                sp_f = spool.tile([128, hc], F32, tag="sp")
                sn_f = spool.tile([128, hc], F32, tag="sn")
                sp = sp_f[:, :hcb]
                sn = sn_f[:, :hcb]

                # ---- fc1 (fp8 DoubleRow) + |.|-sum consumers ----
                # fc1 output goes into 2-bank PAIR tiles so the DVE can
                # reduce both tiles of a pair in ONE 3D tensor_reduce per
                # sign block (per-tile sums land in adjacent sp/sn columns).
                def emit_fc1(j, fc1out):
                    vt4j = vt41[:, j] if j < NB1 else vt42[:, j - NB1]
                    for pr in range(DC // 2):
                        nc.tensor.matmul(
                            out=fc1out,
                            lhsT=vt4j[:, 2 * pr:2 * pr + 2, :],
                            rhs=w3[:, 2 * pr:2 * pr + 2, :],
                            start=(pr == 0),
                            stop=(pr == DC // 2 - 1) and not has_bias,
                            perf_mode=DR,
                        )
                    if has_bias:
                        nc.tensor.matmul(
                            out=fc1out, lhsT=ones_row, rhs=bias_sb,
                            start=False, stop=True,
                        )

                def act_tile(fc1t, j):
                    # both sign blocks of one tile on ACT (2 ops + 2 RAs)
                    for lo, wdt, dst in ((0, hp, sp), (hp, HID - hp, sn)):
                        scra = scrpool.tile([128, 512], F32, tag="scra")
                        nc.scalar.activation(
                            out=scra[:, :wdt], in_=fc1t[:, lo:lo + wdt],
                            func=ACTF.Abs, accum_out=dst[:, j:j + 1],
                        )
                        eng_t["act"] += act_half_cost(wdt)

                def dve_tile(fc1t, j):
                    for lo, wdt, dst in ((0, hp, sp), (hp, HID - hp, sn)):
                        nc.vector.tensor_reduce(
                            dst[:, j:j + 1], fc1t[:, lo:lo + wdt],
                            axis=AXL.X, op=ALU.add,
                            apply_absolute_value=True,
                        )
                        eng_t["dve"] += dve_half_cost(wdt)

                def dve_pair(fc1p, j):
                    # batched: both tiles' block-sums in one op per block
                    for lo, wdt, dst in ((0, hp, sp), (hp, HID - hp, sn)):
                        nc.vector.tensor_reduce(
                            dst[:, j:j + 2], fc1p[:, :, lo:lo + wdt],
                            axis=AXL.X, op=ALU.add,
                            apply_absolute_value=True,
                        )
                        eng_t["dve"] += (120 + 2 * wdt) / 0.96 + 90

                def startup_loads(j):
                    nonlocal vn
                    if bi == 0:
                        if j == 2 and 2 < BPC:
                            pend_vw.append(emit_vw(2))
                        elif j == 3:
                            vn = emit_vn(0, gate=sp_f)

                for j in range(hcb):
                    fc1 = fc1ps.tile([128, HID], F32, tag="fc1")
                    emit_fc1(j, fc1)
                    startup_loads(j)
                    # two |.|-sums (pos-w2 block, neg-w2 block), each ONE
                    # fused instruction on the less-loaded engine
                    for lo, wdt, dst in ((0, hp, sp), (hp, HID - hp, sn)):
                        if eng_t["act"] + act_half_cost(wdt) <= \
                                eng_t["dve"] + dve_half_cost(wdt):
                            scra = scrpool.tile([128, 512], F32, tag="scra")
                            nc.scalar.activation(
                                out=scra[:, :wdt], in_=fc1[:, lo:lo + wdt],
                                func=ACTF.Abs, accum_out=dst[:, j:j + 1],
                            )
                            eng_t["act"] += act_half_cost(wdt)
                        else:
                            nc.vector.tensor_reduce(
                                dst[:, j:j + 1], fc1[:, lo:lo + wdt],
                                axis=AXL.X, op=ALU.add,
                                apply_absolute_value=True,
                            )
                            eng_t["dve"] += dve_half_cost(wdt)

                # ---- batch bi-1's pass-2 on PE now: its alpha has long
                # been ready, so the PE never stalls on scores.  The merge
                # of the DVE-side partial (dacc) waits one MORE batch so the
                # PE never stalls on the slower DVE chain either. ----
                new_defer2 = None
                if defer1 is not None:
                    p_alpha, p_dacc, p_pvn, p_bi, p_hcb, p_nd = defer1
                    pacc = accps.tile([1, D], F32, tag="pacc")
                    for j in range(p_hcb - p_nd):
                        nc.tensor.matmul(
                            out=pacc,
                            lhsT=p_alpha[:, j:j + 1],
                            rhs=p_pvn[:, j * D:(j + 1) * D],
                            start=(j == 0),
                            stop=(p_nd == 0 and j == p_hcb - 1),
                        )
                    tot_f = totps.tile([1, hc], F32, tag="tot")
                    tot = tot_f[:, :p_hcb]
                    nc.tensor.matmul(out=tot, lhsT=ones_col,
                                     rhs=p_alpha[:, :p_hcb],
                                     start=True, stop=True)
                    ro = p_bi * (D + hc)
                    nc.vector.tensor_copy(
                        oball[:, ro + D:ro + D + p_hcb], tot)
                    eng_t["dve"] += 134
                    new_defer2 = (pacc, p_dacc, p_bi, p_hcb, p_nd)
                if defer2 is not None:
                    f_pacc, f_dacc, f_bi, f_hcb, f_nd = defer2
                    if f_nd > 0:
                        nc.tensor.matmul(out=f_pacc, lhsT=ones_col,
                                         rhs=f_dacc,
                                         start=(f_hcb == f_nd), stop=True)
                    ro = f_bi * (D + hc)
                    nc.scalar.copy(oball[:, ro:ro + D], f_pacc)
                    eng_t["act"] += 578
                    nc.gpsimd.dma_start(
                        out=OUT.ap()[f_bi].rearrange("(o x) -> o x", o=1),
                        in_=oball[:, ro:ro + D + hc])

                # ---- scores -> biased -> exp (scale undoes 2*SV*SW) ----
                sc_f = spool.tile([128, hc], F32, tag="sc")
                sc = sc_f[:, :hcb]
                nc.vector.tensor_sub(sc, sp, sn)
                eng_t["dve"] += 70
                scm_f = spool.tile([128, hc], F32, tag="scm")
                scm = scm_f[:, :hcb]
                nc.vector.tensor_add(scm, sc, mb)
                eng_t["dve"] += 70
                alpha32_f = spool.tile([128, hc], F32, tag="alpha32")
                alpha32 = alpha32_f[:, :hcb]
                nc.scalar.activation(
                    out=alpha32, in_=scm, func=ACTF.Exp,
                    bias=float(b2eff), scale=float(escale2),
                )
                eng_t["act"] += 200
                alpha_f = spool.tile([128, hc], F16, tag="alpha")
                alpha = alpha_f[:, :hcb]
                # fp16 cast on ACT: same engine as the exp, so the PE's
                # deferred pass-2 waits on one queue hop instead of two
                nc.scalar.copy(alpha, alpha32)
                eng_t["act"] += 160


                # ---- DVE takes the last ndve pass-2 tiles off the PE;
                # the last batch keeps its chain short (it is on the tail
                # critical path) ----
                ndve_bi = 0 if bi == BPC - 1 else ndve
                if ndve_bi > 0:
                    dacc = spool.tile([128, D], F16, tag="dacc")
                    dacc2 = spool.tile([128, D], F16, tag="dacc2")
                    j0 = hcb - ndve_bi
                    nc.vector.tensor_scalar_mul(
                        dacc, vn[:, j0 * D:(j0 + 1) * D], alpha32[:, j0:j0 + 1])
                    eng_t["dve"] += 194
                    for j in range(j0 + 1, hcb):
                        nc.vector.scalar_tensor_tensor(
                            out=dacc2, in0=vn[:, j * D:(j + 1) * D],
                            scalar=alpha32[:, j:j + 1], in1=dacc,
                            op0=ALU.mult, op1=ALU.add,
                        )
                        eng_t["dve"] += 745
                        dacc, dacc2 = dacc2, dacc
                else:
                    dacc = None

                prev_sp = sp_f
                defer2 = new_defer2
                defer1 = (alpha, dacc, vn, bi, hcb, ndve_bi)

            # tail: batch BPC-2's merge, then batch BPC-1's full pass-2
            if defer2 is not None:
                f_pacc, f_dacc, f_bi, f_hcb, f_nd = defer2
                if f_nd > 0:
                    nc.tensor.matmul(out=f_pacc, lhsT=ones_col, rhs=f_dacc,
                                     start=(f_hcb == f_nd), stop=True)
                ro = f_bi * (D + hc)
                nc.scalar.copy(oball[:, ro:ro + D], f_pacc)
                nc.gpsimd.dma_start(
                    out=OUT.ap()[f_bi].rearrange("(o x) -> o x", o=1),
                    in_=oball[:, ro:ro + D + hc])
            p_alpha, p_dacc, p_pvn, p_bi, p_hcb, p_nd = defer1
            pacc = accps.tile([1, D], F32, tag="pacc")
            for j in range(p_hcb - p_nd):
                nc.tensor.matmul(
                    out=pacc,
                    lhsT=p_alpha[:, j:j + 1],
                    rhs=p_pvn[:, j * D:(j + 1) * D],
                    start=(j == 0),
                    stop=(p_nd == 0 and j == p_hcb - 1),
                )
            if p_nd > 0:
                nc.tensor.matmul(out=pacc, lhsT=ones_col, rhs=p_dacc,
                                 start=(p_hcb == p_nd), stop=True)
            tot_f = totps.tile([1, hc], F32, tag="tot")
            tot = tot_f[:, :p_hcb]
            nc.tensor.matmul(out=tot, lhsT=ones_col, rhs=p_alpha[:, :p_hcb],
                             start=True, stop=True)
            po = p_bi * (D + hc)
            nc.scalar.copy(oball[:, po:po + D], pacc)
            nc.vector.tensor_copy(oball[:, po + D:po + D + p_hcb], tot)
            nc.gpsimd.dma_start(
                out=OUT.ap()[p_bi].rearrange("(o x) -> o x", o=1),
                in_=oball[:, po:po + D + hc])

    nc.finalize()
    return nc


def _prep(K, V, mask, W, w1, b1, w2, b2):
    """Host-side input marshalling (no device work)."""
    import ml_dtypes

    F8NP = ml_dtypes.float8_e4m3
    F8NNP = ml_dtypes.float8_e3m4
    E3MAX = 15.5

    K = np.asarray(K, dtype=np.float32)
    V = np.asarray(V, dtype=np.float32)
    mask = np.asarray(mask).astype(bool)
    W = np.asarray(W, dtype=np.float32)
    w1 = np.asarray(w1, dtype=np.float32)
    b1 = np.asarray(b1, dtype=np.float32)
    w2 = np.asarray(w2, dtype=np.float32).reshape(-1)
    b2 = np.asarray(b2, dtype=np.float32).reshape(-1)

    g = np.diagonal(W).astype(np.float32) * K           # [B, D]
    pos = w2 >= 0.0
    perm = np.argsort(~pos, kind="stable")              # positives first
    hp = int(pos.sum())
    wabs = w1[:, perm] * np.abs(w2[perm])[None, :]      # [D, HID] f32

    # global power-of-2 fp8 scale for the gated weights
    w12 = g[:, :, None] * wabs[None]                    # [B, D, HID]
    wmax = float(np.abs(w12).max()) + 1e-30
    SW = float(2.0 ** np.floor(np.log2(WTARGET / wmax)))
    s2 = 2.0 * SV * SW
    escale2 = 1.0 / s2

    bias12 = (b1[perm] * np.abs(w2[perm])).astype(np.float32) * (SV * SW)
    has_bias = bool(np.any(bias12 != 0.0))
    b2eff = (float(b2[0]) if b2.size else 0.0) + 0.5 * float(b1 @ w2)

    # gated weights, partition-major [128, (c, n)], d = c*128 + p
    WG = np.clip(w12 * SW, -240.0, 240.0).astype(F8NP)
    WG = np.ascontiguousarray(
        WG.reshape(B, DC, 128, HID).transpose(0, 2, 1, 3).reshape(B, 128, DC * HID)
    )

    # exact half-linear score term: 0.5 * sum_n w2_n * pre_n  (per token)
    wsum = (w1.astype(np.float64) @ w2.astype(np.float64))       # [D]
    gl = g.astype(np.float64) * wsum[None, :]                    # [B, D]

    # mask compaction: keep only unmasked tokens, pad to a tile multiple.
    valid = ~mask                                       # [B, H]
    cnt = valid.sum(axis=1)
    order = np.argsort(-cnt, kind="stable")             # descending
    hcs = [max(1, int(-(-int(cnt[order[i * NCORES]]) // 128)))
           for i in range(BPC)]
    hc = hcs[0]

    VW = np.zeros((NCORES, BPC, 128, DC * HID + hc * DC * 128), dtype=F8NP)
    VN = np.zeros((NCORES, BPC, 128, hc * D), dtype=F8NNP)
    MB = np.empty((NCORES, BPC, 128, hc), dtype=np.float32)
    for i in range(BPC):
        hcb = hcs[i]
        HCb = hcb * 128
        tok_pj = np.arange(HCb).reshape(hcb, 128).T     # [p, j] -> token idx
        vtb = np.zeros((D, HCb), dtype=np.float32)
        vnb = np.zeros((HCb, D), dtype=np.float32)
        linb = np.full((HCb,), MASK_FILL * s2, dtype=np.float64)
        for c in range(NCORES):
            b = int(order[i * NCORES + c])
            n = int(cnt[b])
            vb = V[b, valid[b]]                         # [n, D] f32
            vtb[:, :n] = vb.T
            vtb[:, n:] = 0
            # [d=(c,p), t=(j,m)] -> [p, (j, c, m)]  (DoubleRow pair layout)
            VW[c, i, :, :DC * HID] = WG[b]
            VW[c, i, :, DC * HID:DC * HID + hcb * DC * 128] = (
                np.clip(vtb * SV, -240.0, 240.0)
                .reshape(DC, 128, hcb, 128).transpose(1, 2, 0, 3)
                .reshape(128, hcb * DC * 128).astype(F8NP)
            )
            vnb[:n] = vb
            vnb[n:] = 0
            # [t=(j,p), d] -> [p, (j, d)]
            VN[c, i, :, :hcb * D] = (
                np.clip(vnb * SVN, -E3MAX, E3MAX).astype(F8NNP)
                .reshape(hcb, 128, D).transpose(1, 0, 2).reshape(128, hcb * D)
            )
            # additive bias: s2 * (0.5*lin) for real tokens, huge-neg for pads
            linb[:n] = s2 * 0.5 * (vb.astype(np.float64) @ gl[b])
            linb[n:] = MASK_FILL * s2
            MB[c, i, :, :hcb] = linb[tok_pj].astype(np.float32)
            MB[c, i, :, hcb:] = 0.0

    return (VW, VN, MB, bias12, has_bias, hcs, hp, b2eff, escale2, order)


def _compile_and_maps(**inputs):
    VW, VN, MB, bias12, has_bias, hcs, hp, b2eff, escale2, order = _prep(**inputs)
    nc = _build(hcs, hp, b2eff, has_bias, escale2)
    in_maps = []
    for c in range(NCORES):
        m = {"VW": VW[c], "VN": VN[c], "MB": MB[c]}
        if has_bias:
            m["BI"] = bias12.reshape(1, HID)
        in_maps.append(m)
    return nc, in_maps, order


def kernel(K, V, mask, W, w1, b1, w2, b2):
    from concourse import bass_utils

    nc, in_maps, order = _compile_and_maps(
        K=K, V=V, mask=mask, W=W, w1=w1, b1=b1, w2=w2, b2=b2
    )
    res = bass_utils.run_bass_kernel_spmd(nc, in_maps, core_ids=list(range(NCORES)))
    out = np.empty((B, D), dtype=np.float32)
    for c in range(NCORES):
        ot = res.results[c]["OUT"].astype(np.float64)        # [BPC, D+hc]
        po = ot[:, :D]
        norm = SVN * ot[:, D:].sum(axis=1)                   # [BPC]
        for i in range(BPC):
            out[int(order[i * NCORES + c])] = (po[i] / norm[i]).astype(np.float32)
    return out
